# revision 79
# baseline (speedup 1.0000x reference)
"""Trainium2 Bass kernel for ExternalEmbeddingSelfAttention.

Computation (per batch b):
    q     = hs @ Wq + bq           [S,H]
    k_tok = hs @ Wk + bk           [S,H]
    v_tok = hs @ Wv + bv           [S,H]
    k_ext = ext @ Wk + bk          [E,H]
    v_ext = ext @ Wv + bv          [E,H]
    s_self[t] = q[t] . k_tok[t]                (per-token self score)
    s_ext = q @ k_ext^T            [S,E]
    probs = softmax([s_self, s_ext], axis=-1)  (no 1/sqrt(d) scaling)
    out   = probs[:,0:1]*v_tok + probs[:,1:] @ v_ext

Score-path folding (the big win vs the v0 kernel): softmax is invariant
to a per-row (per-token) shift, so with M = Wq Wk^T and w = Wk bq folded
on the host,
    A''[t]    = hs[t] @ M + w                       [T,H]
    s_ext'    = A'' @ ext^T  (= s_ext - col(t) - bq.bk)
    s_self'   = A''[t] . hs[t]  (= s_self - col(t) - bq.bk)
(both shifted by the same per-token constant col(t) = hs[t].(Wq bk) plus
the global constant bq.bk, which softmax cancels).  The device never
computes k_tok / k_ext: the f32r score path drops from 196k to ~102k PE
cycles.  s_self' is per-chunk products on DVE (dke = A''^T (.) xT) column
-summed by tiny PE ones-matmuls accumulated in one PSUM tile.

Sharding: 8 cores, each takes 1024 contiguous tokens of the flattened
(B*S, H) token axis (core i -> batch i//2, S-half i%2).  Each core also
computes its batch's external V projections (duplicated across the 2
cores sharing a batch).

Device schedule (per core, T=1024): DMA in first-needed-first order
(head slices spread over the sync/Act/SWDGE queues so their fixed
latencies overlap).  xT8 (the fp8 hi/lo split of xT feeding the v_tok
matmuls) is computed ON DEVICE on Act+Pool while xT streams in, saving
2MB of DMA.  PE phases: QA, QB (A''^T = M^T@xT; psq readout split
Act/DVE, monoB strips first so V0's banks free early; dke = QT (.) xT
on DVE/Pool), V0, sspA/sspB (s_self column sums, split per ho-half so
each only waits its own readout), V1, then attention per 128-token tile
(s_ext = A''T.T@eT f32r; softmax with self column; probs PE-transposed
into ONE PSUM tile, fp8-split hi on Act / lo on DVE; ctx accumulates
Pt.T@Vx fp8-DR, the unscaled v_tok scaled at readout via an Act
per-partition-scale copy + Pool bias add + DVE ctx fold).

PSUM rules honoured (the HW verifier rejects what CoreSim allows):
GPSIMD never touches PSUM; DVE ops read at most one PSUM operand; a
PSUM accumulation group is always a whole tile (never column slices).
PSUM is split into two 4-bank pools (monoA: psq[0:4]+ssp, monoB:
psq[4:8]+psv) so the attention pools ps_att/ps_tr (which land on monoA's
banks) only wait on monoA's release: tile 0's s_ext/softmax overlaps the
V1 readout ladder that gates monoB/ps_cu.

Wv/Vx are fp8 hi/lo (x32 host pre-scale avoids e4m3 subnormals; 1/32
applied at readout).  Score pipeline stays f32r (FP22) end to end.
"""

import numpy as np
import ml_dtypes

B, S, E, H = 4, 2048, 512, 1024
NCORES = 8
T = (B * S) // NCORES  # tokens per core = 1024

_RUNNER_CACHE = {}

_ONESC = np.ones((128, 2), dtype=np.float32)
_EYEB = np.eye(128, dtype=ml_dtypes.bfloat16)

LN128 = 4.852030263919617  # ln(128): probs are computed x128 for fp8


# --------------------------------------------------------------------------
# device kernel emission
# --------------------------------------------------------------------------

def _emit(nc, tc, ctx, T, H, E, reps=1):
    import contextlib
    import concourse.mybir as mybir

    f32 = mybir.dt.float32
    f32r = mybir.dt.float32r
    bf16 = mybir.dt.bfloat16
    f8 = mybir.dt.float8e4
    DR = mybir.MatmulPerfMode.DoubleRow
    Alu = mybir.AluOpType
    Act = mybir.ActivationFunctionType
    X = mybir.AxisListType.X

    KC = H // 128          # contraction chunks over h_in; also h_out tiles
    NT = T // 128          # token tiles
    NE = E // 128          # ext tiles
    WH = min(512, H)       # h_out free-dim chunk
    NH = H // WH
    WT = min(512, T)       # token free-dim chunk
    NTW = T // WT
    assert E <= 512 and NTW == 2 and NH == 2 and KC == 8

    xT_d = nc.declare_dram_parameter("xT", [H, T], f32, isOutput=False)
    eT_d = nc.declare_dram_parameter("eT", [H, E], f32, isOutput=False)
    eT8h_d = nc.declare_dram_parameter("eT8h", [H // 2, 2 * E], f8,
                                       isOutput=False)
    eT8l_d = nc.declare_dram_parameter("eT8l", [H // 2, 2 * E], f8,
                                       isOutput=False)

    # "Wq" carries the host-folded M = Wq @ Wk^T
    wq_d = nc.declare_dram_parameter("Wq", [H, H], f32, isOutput=False)
    wv8h_d = nc.declare_dram_parameter("Wv8h", [H // 2, 2 * H], f8,
                                       isOutput=False)
    wv8l_d = nc.declare_dram_parameter("Wv8l", [H // 2, 2 * H], f8,
                                       isOutput=False)
    # "bqc" carries the host-folded w = Wk @ bq, chunked [128, KC]
    bqc_d = nc.declare_dram_parameter("bqc", [128, KC], f32, isOutput=False)
    bvr_d = nc.declare_dram_parameter("bvr", [1, H], bf16, isOutput=False)
    onesc_d = nc.declare_dram_parameter("onesc", [128, 2], f32, isOutput=False)
    eye_d = nc.declare_dram_parameter("eye", [128, 128], bf16, isOutput=False)
    out_d = nc.declare_dram_parameter("out", [T, H], f32, isOutput=True)

    cp = ctx.enter_context(tc.tile_pool(name="cp", bufs=1))
    small = ctx.enter_context(tc.tile_pool(name="small", bufs=3))

    # ---- persistent SBUF tensors -----------------------------------------
    QT = cp.tile([128, KC * T], f32r, name="QT")    # A''^T hidden-major
    dke = cp.tile([128, KC * T], f32r, name="dke")  # per-chunk A''*x prods
    Vx8h = cp.tile([128, NE * H], f8, name="Vx8h")  # V_ext E-major fp8 hi
    Vx8l = cp.tile([128, NE * H], f8, name="Vx8l")  # and lo residual
    xT = cp.tile([128, KC * T], f32r, name="xT")    # chunk kc: cols [kc*T,+T)
    eT = cp.tile([128, KC * E], f32r, name="eT")
    NP = KC // 2  # DoubleRow kc-pairs
    Wv8h = cp.tile([128, NP * 2 * H], f8, name="Wv8h")  # 32*Wv hi/lo fp8
    Wv8l = cp.tile([128, NP * 2 * H], f8, name="Wv8l")
    eT8h = cp.tile([128, NP * 2 * E], f8, name="eT8h")
    eT8l = cp.tile([128, NP * 2 * E], f8, name="eT8l")
    xT8h = cp.tile([128, NP * 2 * T], f8, name="xT8h")
    xT8l = cp.tile([128, NP * 2 * T], f8, name="xT8l")
    ss_col = cp.tile([128, NT], f32, name="ss_col")    # s_self per tile
    ssm_col = cp.tile([128, NT], f32, name="ssm_col")  # ln128 - s_self
    bqc = cp.tile([128, KC], f32, name="bqc")       # holds w = Wk bq chunks
    bvr = cp.tile([1, H], bf16, name="bvr")
    bvb = cp.tile([128, H], bf16, name="bvb")
    ones_c = cp.tile([128, 2], f32r, name="ones_c")
    identb = cp.tile([128, 128], bf16, name="identb")

    loop_cm = tc.For_i(0, reps, 1) if reps > 1 else contextlib.nullcontext()
    with loop_cm:
      with tc.tile_pool(name="monoA", bufs=4, space="PSUM") as monoA, \
           tc.tile_pool(name="monoB", bufs=4, space="PSUM") as monoB:
        with tc.tile_pool(name="wq", bufs=4) as wq_pool:
            wqa = [wq_pool.tile([128, WT], f32r, name="wqa") for _ in range(KC)]
            wqb = [wq_pool.tile([128, WT], f32r, name="wqb") for _ in range(KC)]

            # ---- the ordered input stream (single queue = device order) --
            # wqa[0] and a 256-col slice of xT[0] lead, spread over three
            # queues so their fixed DMA latencies overlap: the first
            # matmul only waits on ~384KB.
            nc.sync.dma_start(wqa[0][:, 0:128],
                              wq_d[0:128, 0:128].bitcast(f32r))
            nc.scalar.dma_start(xT[:, 0:256],
                                xT_d[0:128, 0:256].bitcast(f32r))
            nc.gpsimd.dma_start(xT[:, 256:WT],
                                xT_d[0:128, 256:WT].bitcast(f32r))
            nc.sync.dma_start(wqa[0][:, 128:WT],
                              wq_d[0:128, 128:WT].bitcast(f32r))
            nc.sync.dma_start(xT[:, WT:T], xT_d[0:128, WT:T].bitcast(f32r))
            for kc in range(1, KC):
                nc.sync.dma_start(xT[:, kc * T:(kc + 1) * T],
                                  xT_d[kc * 128:(kc + 1) * 128, :].bitcast(f32r))
                nc.sync.dma_start(
                    wqa[kc][:],
                    wq_d[kc * 128:(kc + 1) * 128, 0:WT].bitcast(f32r))
                if kc == 2:
                    nc.sync.dma_start(bqc[:], bqc_d[:])
            for kc in range(KC):
                nc.sync.dma_start(
                    wqb[kc][:],
                    wq_d[kc * 128:(kc + 1) * 128, WT:H].bitcast(f32r))
            nc.sync.dma_start(ones_c[:], onesc_d[:].bitcast(f32r))
            nc.sync.dma_start(bvr[:], bvr_d[:])
            nc.sync.dma_start(identb[:], eye_d[:])
            for pr in range(NP):
                nc.sync.dma_start(Wv8h[:, pr * 2 * H:(pr + 1) * 2 * H],
                                  wv8h_d[pr * 128:(pr + 1) * 128, :])
                nc.sync.dma_start(Wv8l[:, pr * 2 * H:(pr + 1) * 2 * H],
                                  wv8l_d[pr * 128:(pr + 1) * 128, :])
                nc.sync.dma_start(eT8h[:, pr * 2 * E:(pr + 1) * 2 * E],
                                  eT8h_d[pr * 128:(pr + 1) * 128, :])
                nc.sync.dma_start(eT8l[:, pr * 2 * E:(pr + 1) * 2 * E],
                                  eT8l_d[pr * 128:(pr + 1) * 128, :])
            for kc in range(KC):
                nc.sync.dma_start(eT[:, kc * E:(kc + 1) * E],
                                  eT_d[kc * 128:(kc + 1) * 128, :].bitcast(f32r))

            # preload the Exp activation table off the critical path
            dummy = small.tile([1, 2], f32, name="dummy")
            nc.scalar.memzero(dummy[:])
            nc.scalar.activation(dummy[:], dummy[:], Act.Exp, bias=0.0,
                                 scale=1.0)
            # bvb = bv broadcast to all partitions (Pool; no PE/PSUM)
            nc.gpsimd.partition_broadcast(bvb[:], bvr[:])
            # xT8 hi/lo fp8 split computed on-device (Act/Pool ride the
            # otherwise-idle window while xT streams in) instead of 2MB
            # of DMA; the psu matmuls then never race the DMA tail
            for kc in range(KC):
                o = (kc // 2) * 2 * T + (kc % 2) * T
                sl = slice(kc * T, (kc + 1) * T)
                nc.scalar.copy(xT8h[:, o:o + T], xT[:, sl])
                nc.gpsimd.tensor_tensor(xT8l[:, o:o + T], xT[:, sl],
                                        xT8h[:, o:o + T], Alu.subtract)

            # ---- QA / QB: A''^T = M^T @ xT (+w), kc-outer, half-ho -------
            # psq readout runs on the Act engine (Identity + per-partition
            # w bias); the dke product (A''^T (.) xT, feeds s_self) on DVE.
            for half, wqs in ((0, wqa), (1, wqb)):
                psq = ([monoA.tile([128, WT], f32, name="psqA", tag="acc")
                        for _ in range(4)]
                       + [monoB.tile([128, WT], f32, name="psqB", tag="acc")
                          for _ in range(4)])
                for kc in range(KC):
                    for n in range(NTW):
                        for ho4 in range(4):
                            nc.tensor.matmul(
                                psq[ho4 * NTW + n][:],
                                wqs[kc][:, ho4 * 128:(ho4 + 1) * 128],
                                xT[:, kc * T + n * WT: kc * T + (n + 1) * WT],
                                start=(kc == 0), stop=(kc == KC - 1))
                # readout: each psq bank has ONE reader (the QT add, split
                # Act/DVE so the ladder halves and banks free fast, monoB
                # strips first so V0 unblocks early); the dke product then
                # reads QT from SBUF (DVE/Pool) without holding banks
                for s in (4, 5, 6, 7, 0, 1, 2, 3):
                    ho4, n = s // NTW, s % NTW
                    ho = half * 4 + ho4
                    sl = slice(ho * T + n * WT, ho * T + (n + 1) * WT)
                    if s % 2 == 0:
                        nc.scalar.add(QT[:, sl], psq[s][:],
                                      bqc[:, ho:ho + 1])
                    else:
                        nc.vector.tensor_scalar_add(QT[:, sl], psq[s][:],
                                                    bqc[:, ho:ho + 1])
                for s in (4, 5, 6, 7, 0, 1, 2, 3):
                    ho4, n = s // NTW, s % NTW
                    ho = half * 4 + ho4
                    sl = slice(ho * T + n * WT, ho * T + (n + 1) * WT)
                    eng = nc.vector if s % 2 == 0 else nc.gpsimd
                    eng.tensor_tensor(dke[:, sl], QT[:, sl], xT[:, sl],
                                      Alu.mult)

        # ---- V_ext: Vx = eT^T @ Wv via fp8 DoubleRow; Wv is pre-scaled
        # x32 on the host so its hi/lo fp8 split avoids e4m3 subnormals,
        # and the 1/32 is applied at readout.  3 cross terms; the lo*lo
        # term (~0.4%) is dropped.  eblk 0 runs right after QB; the ssp
        # column sums fill the gap before eblk 1. ------------------------
        def pair2(t, pr, width, lo, hi):
            return t[:, pr * 2 * width:(pr + 1) * 2 * width].rearrange(
                "p (two w) -> p two w", two=2)[:, :, lo:hi]

        VTERMS = ((eT8h, Wv8h), (eT8h, Wv8l), (eT8l, Wv8h))

        def v_ext_eblk(eblk, defer=None):
            psv = [monoB.tile([128, WH], f32, name="psv", tag="acc")
                   for _ in range(4)]
            for pr in range(NP):
                for ti, (lt, rt) in enumerate(VTERMS):
                    for e2 in range(2):
                        eo = eblk * 2 + e2
                        for n in range(NH):
                            nc.tensor.matmul(
                                psv[e2 * NH + n][:],
                                pair2(lt, pr, E, eo * 128, (eo + 1) * 128),
                                pair2(rt, pr, H, n * WH, (n + 1) * WH),
                                start=(pr == 0 and ti == 0),
                                stop=(pr == NP - 1 and ti == len(VTERMS) - 1),
                                perf_mode=DR)
            for e2 in range(2):
                eo = eblk * 2 + e2
                for n in range(NH):
                    s = e2 * NH + n
                    ph = Vx8h[:, eo * H + n * WH: eo * H + (n + 1) * WH]
                    pl = Vx8l[:, eo * H + n * WH: eo * H + (n + 1) * WH]
                    if defer is None:
                        nc.scalar.activation(ph, psv[s][:], Act.Copy,
                                             bias=0.0, scale=1.0 / 32)
                        nc.vector.scalar_tensor_tensor(
                            pl, psv[s][:], 1.0 / 32, ph,
                            Alu.mult, Alu.subtract)
                    else:
                        # decouple the fp8 split from the PSUM banks (so
                        # the release does not sit on the DVE queue right
                        # when tile 0's softmax chain needs it): a single
                        # Act copy per bank to an f32 SBUF stage, the
                        # hi/lo split deferred into the attention window
                        vb = vbs[s]
                        nc.scalar.mul(vb[:], psv[s][:], 1.0 / 32)
                        defer.append((vb, ph, pl))

        # s_self: column-sum dke via tiny matmuls, one PSUM tile per
        # (ho-half, token tile) so each accumulation group is a whole
        # tile; the early half only waits on its own readout ladder and
        # the halves are combined on DVE during V1.
        def ssp_half(h0, consume):
            for m in range(NT):
                ssp = monoA.tile([128, 2], f32, name="ssp", tag="acc")
                for kc in range(h0, h0 + KC // 2):
                    nc.tensor.matmul(
                        ssp[:],
                        dke[:, kc * T + m * 128:kc * T + (m + 1) * 128],
                        ones_c[:], start=(kc == h0),
                        stop=(kc == h0 + KC // 2 - 1))
                consume(m, ssp)

        ssp_half(0, lambda m, ssp: nc.vector.tensor_copy(
            ss_col[:, m:m + 1], ssp[:, 0:1]))

        v_ext_eblk(0)

        ssp_half(KC // 2, lambda m, ssp: nc.vector.tensor_tensor(
            ss_col[:, m:m + 1], ss_col[:, m:m + 1], ssp[:, 0:1], Alu.add))
        # ssm = ln128 - s_self (pre-folded for the per-tile softmax max)
        nc.vector.tensor_scalar(ssm_col[:], ss_col[:], -1.0, LN128,
                                Alu.mult, Alu.add)

        v_ext_eblk(1)

      # ---- attention per token tile ------------------------------------
      # v_tok runs UNSCALED (xTb @ Wv, no softmax dependency) in its own
      # PSUM groups, filling the PE while the softmax chain computes; the
      # p_self scaling is applied per-partition at readout.
      with tc.tile_pool(name="ps_att", bufs=2, space="PSUM") as ps_att, \
           tc.tile_pool(name="ps_tr", bufs=2, space="PSUM") as ps_tr, \
           tc.tile_pool(name="ps_cu", bufs=1, space="PSUM") as ps_cu, \
           tc.tile_pool(name="work_a", bufs=4) as work_a, \
           tc.tile_pool(name="pt8", bufs=2) as pt8_pool:
        for m in range(NT):
            last = m == NT - 1
            # s_ext = A''^T.T @ eT  -> [128 tokens, E]  (f32r)
            ps_s = ps_att.tile([128, E], f32, name="ps_s")
            for kc in range(KC):
                nc.tensor.matmul(
                    ps_s[:],
                    QT[:, kc * T + m * 128: kc * T + (m + 1) * 128],
                    eT[:, kc * E:(kc + 1) * E],
                    start=(kc == 0), stop=(kc == KC - 1))

            nmx = small.tile([128, 1], f32, name="nmx")
            nc.vector.tensor_reduce(nmx[:], ps_s[:], axis=X, op=Alu.max,
                                    negate=True)
            # nmx2b = min(ln128 - ss, ln128 - max(s_ext))
            #       = ln128 - max(ss, max(s_ext))
            nmx2b = small.tile([128, 1], f32, name="nmx2b")
            nc.vector.scalar_tensor_tensor(
                nmx2b[:], nmx[:], LN128, ssm_col[:, m:m + 1],
                Alu.add, Alu.min)

            # probs are computed x128 (bias includes ln128) so their fp8
            # hi/lo split stays clear of e4m3 subnormals; the 1/128 is
            # carried by r' = 1/(128 Z).  exp runs in two 256-col halves
            # so the transposes/fp8-split/ctx start on half the tile.
            pe = work_a.tile([128, E], bf16, name="pe")
            Ze = small.tile([128, 1], f32, name="Ze")
            nc.scalar.activation(pe[:], ps_s[:], Act.Exp, bias=nmx2b[:],
                                 scale=1.0, accum_out=Ze[:])
            # P' = 128 * p_self
            p128 = small.tile([128, 1], f32, name="p128")
            nc.scalar.activation(p128[:], ss_col[:, m:m + 1],
                                 Act.Exp, bias=nmx2b[:], scale=1.0)
            Zt = small.tile([128, 1], f32, name="Zt")
            nc.vector.tensor_tensor(Zt[:], Ze[:], p128[:], Alu.add)
            rp = small.tile([128, 1], f32, name="rp")
            nc.vector.reciprocal(rp[:], Zt[:])
            pr = small.tile([128, 1], f32, name="pr")
            nc.vector.scalar_tensor_tensor(pr[:], p128[:], 1.0 / 32,
                                           rp[:], Alu.mult, Alu.mult)

            # unscaled v_tok: psu[n] = (xTb slice).T @ Wv — independent of
            # the softmax, keeps the PE busy during the chain above.
            # The probability transposes are emitted MID-psu (exp is
            # ready by then) so the trailing psu matmuls cover the fp8
            # conversion latency instead of the PE idling before ctx.
            psu = [ps_cu.tile([128, WH], f32, name=f"psu{n}",
                              tag=f"u{n}") for n in range(NH)]
            pst = ps_tr.tile([128, NE * 128], bf16, name="pst")
            UTERMS = ((xT8h, Wv8h), (xT8h, Wv8l), (xT8l, Wv8h))
            for pr8 in range(NP):
                for ti, (lt, rt) in enumerate(UTERMS):
                    if pr8 == 2 and ti == 2:
                        # transpose unnormalized ext probs into ONE PSUM
                        # tile (disjoint column slices run back to back)
                        for ec in range(NE):
                            nc.tensor.transpose(
                                pst[:, ec * 128:(ec + 1) * 128],
                                pe[:, ec * 128:(ec + 1) * 128],
                                identb[:])
                    lhsT = pair2(lt, pr8, T, m * 128, (m + 1) * 128)
                    for n in range(NH):
                        nc.tensor.matmul(
                            psu[n][:], lhsT,
                            pair2(rt, pr8, H, n * WH, (n + 1) * WH),
                            start=(pr8 == 0 and ti == 0),
                            stop=(pr8 == NP - 1 and
                                  ti == len(UTERMS) - 1),
                            perf_mode=DR)
            # (GPSIMD cannot read PSUM on hardware: hi split on Act, lo
            # residual on DVE with the single allowed PSUM operand; the
            # last tile puts both on DVE so its ctx is not stuck behind
            # the Act queue)
            Pt8h = pt8_pool.tile([128, NE * 128], f8, name="Pt8h")
            Pt8l = pt8_pool.tile([128, NE * 128], f8, name="Pt8l")
            (nc.vector.tensor_copy if last else nc.scalar.copy)(
                Pt8h[:], pst[:])
            nc.vector.tensor_tensor(Pt8l[:], pst[:], Pt8h[:], Alu.subtract)

            # ctx_ext = Pt.T @ Vx
            psc = [ps_cu.tile([128, WH], f32, name=f"psc{n}", tag=f"c{n}")
                   for n in range(NH)]
            CTERMS = ((Pt8h, Vx8h), (Pt8h, Vx8l), (Pt8l, Vx8h))
            NEP = NE // 2  # ec-pairs

            # (ep, ti) visit order: the h0-fed groups first, the Pt8l-fed
            # term (ti=2) last, matching when each fp8 half lands
            CTX_ORDER = ((0, 0), (0, 1), (1, 0), (1, 1), (0, 2), (1, 2))

            def ctx_mm(n, ep, ti, start, stop):
                lt, rt = CTERMS[ti]
                nc.tensor.matmul(
                    psc[n][:],
                    lt[:].rearrange("p (ep two e) -> p ep two e",
                                    ep=NEP, two=2)[:, ep],
                    rt[:, 2 * ep * H:(2 * ep + 2) * H].rearrange(
                        "p (two h) -> p two h",
                        two=2)[:, :, n * WH:(n + 1) * WH],
                    start=start, stop=stop, perf_mode=DR)

            if not last:
                for i, (ep, ti) in enumerate(CTX_ORDER):
                    for n in range(NH):
                        ctx_mm(n, ep, ti, i == 0, i == len(CTX_ORDER) - 1)
            else:
                # finish chunk 1 first so its readout+store overlaps
                # chunk 0's matmuls and the drain is one chunk shorter
                for n in (1, 0):
                    for i, (ep, ti) in enumerate(CTX_ORDER):
                        ctx_mm(n, ep, ti, i == 0, i == len(CTX_ORDER) - 1)

            # out = r'*ctx_ext + (P'*r'/32)*v_tok + bvb, stored in strips
            order = (1, 0) if last else tuple(range(NH))
            osbs = {}
            # psu stops well before ctx_ext: fold it early, split as
            # Act (x pr, per-partition scale) + Pool (+bvb, all-SBUF),
            # keeping DVE free for the ctx folds
            for n in order:
                osbs[n] = work_a.tile([128, WH], f32, name="osb")
                nc.scalar.activation(osbs[n][:], psu[n][:], Act.Copy,
                                     bias=0.0, scale=pr[:])
                nc.gpsimd.tensor_tensor(
                    osbs[n][:], osbs[n][:],
                    bvb[:, n * WH:(n + 1) * WH], Alu.add)
            for n in order:
                if last and n == 0:
                    # split the final strip so the last store is smaller
                    # and launches off a shorter fold
                    for c0, c1, q in ((128, 512, nc.scalar),
                                      (0, 128, nc.sync)):
                        nc.vector.scalar_tensor_tensor(
                            osbs[n][:, c0:c1], psc[n][:, c0:c1],
                            rp[:], osbs[n][:, c0:c1],
                            Alu.mult, Alu.add)
                        q.dma_start(
                            out_d[m * 128:(m + 1) * 128, c0:c1],
                            osbs[n][:, c0:c1])
                    continue
                nc.vector.scalar_tensor_tensor(
                    osbs[n][:], psc[n][:],
                    rp[:], osbs[n][:],
                    Alu.mult, Alu.add)
                if last:
                    nc.sync.dma_start(
                        out_d[m * 128:(m + 1) * 128,
                              n * WH:n * WH + 256], osbs[n][:, 0:256])
                    nc.scalar.dma_start(
                        out_d[m * 128:(m + 1) * 128,
                              n * WH + 256:(n + 1) * WH],
                        osbs[n][:, 256:512])
                else:
                    nc.scalar.dma_start(
                        out_d[m * 128:(m + 1) * 128, n * WH:(n + 1) * WH],
                        osbs[n][:])


def _build_module(T, H, E, reps=1):
    from contextlib import ExitStack
    import concourse.tile as tile
    from concourse import bacc

    nc = bacc.Bacc(None)
    with ExitStack() as ctx:
        tc = ctx.enter_context(tile.TileContext(nc))
        _emit(nc, tc, ctx, T, H, E, reps)
    nc.finalize()
    return nc


# --------------------------------------------------------------------------
# host side
# --------------------------------------------------------------------------

def _shard_inputs(hidden_states, external_embeddings, Wq, bq, Wk, bk, Wv, bv):
    """Build the per-core input maps (host-side layout prep)."""
    hs = np.asarray(hidden_states, dtype=np.float32)
    ext = np.asarray(external_embeddings, dtype=np.float32)
    Wq64 = np.asarray(Wq, dtype=np.float64)
    Wk64 = np.asarray(Wk, dtype=np.float64)
    Wv = np.asarray(Wv, dtype=np.float32)
    bq = np.asarray(bq, dtype=np.float64)
    bv = np.asarray(bv, dtype=np.float32)

    # score-path weight folding (see module docstring): the device only
    # ever sees M = Wq Wk^T (as "Wq") and w = Wk bq (as "bqc"); the
    # per-token shift hs.(Wq bk) and the constant bq.bk cancel in softmax.
    M = np.ascontiguousarray((Wq64 @ Wk64.T).astype(np.float32))
    wbq = (Wk64 @ bq).astype(np.float32)

    f8 = ml_dtypes.float8_e4m3

    def fp8_pairs(a):
        """[H, N] f32 -> hi/lo fp8 arrays [H//2, 2*N] in DoubleRow
        kc-pair layout: row pr*128+p holds chunks (2pr, 2pr+1)."""
        Hd, N = a.shape
        hi = a.astype(f8)
        lo = (a - hi.astype(np.float32)).astype(f8)
        out = []
        for arr in (hi, lo):
            v = arr.reshape(Hd // 256, 2, 128, N).transpose(0, 2, 1, 3)
            out.append(np.ascontiguousarray(v.reshape(Hd // 2, 2 * N)))
        return out

    Wv8h, Wv8l = fp8_pairs(Wv * 32.0)

    KC = H // 128
    bqc = np.ascontiguousarray(wbq.reshape(KC, 128).T)  # [128, KC]
    bvr = np.ascontiguousarray(bv.reshape(1, H).astype(ml_dtypes.bfloat16))

    flat = hs.reshape(B * S, H)
    in_maps = []
    _ET8 = {}
    for c in range(NCORES):
        b = (c * T) // S
        xT = np.ascontiguousarray(flat[c * T:(c + 1) * T, :].T)  # [H, T]
        eT = np.ascontiguousarray(ext[b].T)                      # [H, E]
        eT8h, eT8l = _ET8.setdefault(b, fp8_pairs(eT))
        in_maps.append({
            "xT": xT,
            "eT": eT, "eT8h": eT8h, "eT8l": eT8l,
            "Wq": M, "Wv8h": Wv8h, "Wv8l": Wv8l,
            "bqc": bqc, "bvr": bvr,
            "onesc": _ONESC, "eye": _EYEB,
        })
    return in_maps


def kernel(hidden_states, external_embeddings, Wq, bq, Wk, bk, Wv, bv):
    from concourse.bass_utils import run_bass_kernel_spmd

    key = "main"
    if key not in _RUNNER_CACHE:
        _RUNNER_CACHE[key] = _build_module(T, H, E)
    nc = _RUNNER_CACHE[key]

    in_maps = _shard_inputs(hidden_states, external_embeddings,
                            Wq, bq, Wk, bk, Wv, bv)
    res = run_bass_kernel_spmd(nc, in_maps, list(range(NCORES)))
    out = np.concatenate([res.results[c]["out"] for c in range(NCORES)],
                         axis=0)
    return out.reshape(B, S, H)


# revision 90
# speedup vs baseline: 1.0127x; 1.0127x over previous
"""Trainium2 Bass kernel for ExternalEmbeddingSelfAttention.

Computation (per batch b):
    q     = hs @ Wq + bq           [S,H]
    k_tok = hs @ Wk + bk           [S,H]
    v_tok = hs @ Wv + bv           [S,H]
    k_ext = ext @ Wk + bk          [E,H]
    v_ext = ext @ Wv + bv          [E,H]
    s_self[t] = q[t] . k_tok[t]                (per-token self score)
    s_ext = q @ k_ext^T            [S,E]
    probs = softmax([s_self, s_ext], axis=-1)  (no 1/sqrt(d) scaling)
    out   = probs[:,0:1]*v_tok + probs[:,1:] @ v_ext

Score-path folding (the big win vs the v0 kernel): softmax is invariant
to a per-row (per-token) shift, so with M = Wq Wk^T and w = Wk bq folded
on the host,
    A''[t]    = hs[t] @ M + w                       [T,H]
    s_ext'    = A'' @ ext^T  (= s_ext - col(t) - bq.bk)
    s_self'   = A''[t] . hs[t]  (= s_self - col(t) - bq.bk)
(both shifted by the same per-token constant col(t) = hs[t].(Wq bk) plus
the global constant bq.bk, which softmax cancels).  The device never
computes k_tok / k_ext: the f32r score path drops from 196k to ~102k PE
cycles.  s_self' is per-chunk products on DVE (dke = A''^T (.) xT) column
-summed by tiny PE ones-matmuls accumulated in one PSUM tile.

Sharding: 8 cores, each takes 1024 contiguous tokens of the flattened
(B*S, H) token axis (core i -> batch i//2, S-half i%2).  Each core also
computes its batch's external V projections (duplicated across the 2
cores sharing a batch).

Device schedule (per core, T=1024): DMA in first-needed-first order
(head slices spread over the sync/Act/SWDGE queues so their fixed
latencies overlap).  xT8 (the fp8 hi/lo split of xT feeding the v_tok
matmuls) is computed ON DEVICE on Act+Pool while xT streams in, saving
2MB of DMA.  PE phases: QA, QB (A''^T = M^T@xT; psq readout split
Act/DVE, monoB strips first so V0's banks free early; dke = QT (.) xT
on DVE/Pool), V0, sspA/sspB (s_self column sums, split per ho-half so
each only waits its own readout), V1, then attention per 128-token tile
(s_ext = A''T.T@eT f32r; softmax with self column; probs PE-transposed
into ONE PSUM tile, fp8-split hi on Act / lo on DVE; ctx accumulates
Pt.T@Vx fp8-DR, the unscaled v_tok scaled at readout via an Act
per-partition-scale copy + Pool bias add + DVE ctx fold).

PSUM rules honoured (the HW verifier rejects what CoreSim allows):
GPSIMD never touches PSUM; DVE ops read at most one PSUM operand; a
PSUM accumulation group is always a whole tile (never column slices).
PSUM is split into two 4-bank pools (monoA: psq[0:4]+ssp, monoB:
psq[4:8]+psv) so the attention pools ps_att/ps_tr (which land on monoA's
banks) only wait on monoA's release: tile 0's s_ext/softmax overlaps the
V1 readout ladder that gates monoB/ps_cu.

Wv/Vx are fp8 hi/lo (x32 host pre-scale avoids e4m3 subnormals; 1/32
applied at readout).  Score pipeline stays f32r (FP22) end to end.
"""

import numpy as np
import ml_dtypes

B, S, E, H = 4, 2048, 512, 1024
NCORES = 8
T = (B * S) // NCORES  # tokens per core = 1024

_RUNNER_CACHE = {}

_ONESC = np.ones((128, 2), dtype=np.float32)
_EYEB = np.eye(128, dtype=ml_dtypes.bfloat16)

LN128 = 4.852030263919617  # ln(128): probs are computed x128 for fp8


# --------------------------------------------------------------------------
# device kernel emission
# --------------------------------------------------------------------------

def _emit(nc, tc, ctx, T, H, E, reps=1):
    import contextlib
    import concourse.mybir as mybir

    f32 = mybir.dt.float32
    f32r = mybir.dt.float32r
    bf16 = mybir.dt.bfloat16
    f8 = mybir.dt.float8e4
    DR = mybir.MatmulPerfMode.DoubleRow
    Alu = mybir.AluOpType
    Act = mybir.ActivationFunctionType
    X = mybir.AxisListType.X

    KC = H // 128          # contraction chunks over h_in; also h_out tiles
    NT = T // 128          # token tiles
    NE = E // 128          # ext tiles
    WH = min(512, H)       # h_out free-dim chunk
    NH = H // WH
    WT = min(512, T)       # token free-dim chunk
    NTW = T // WT
    assert E <= 512 and NTW == 2 and NH == 2 and KC == 8

    xT_d = nc.declare_dram_parameter("xT", [H, T], f32, isOutput=False)
    eT_d = nc.declare_dram_parameter("eT", [H, E], f32, isOutput=False)
    eT8h_d = nc.declare_dram_parameter("eT8h", [H // 2, 2 * E], f8,
                                       isOutput=False)
    eT8l_d = nc.declare_dram_parameter("eT8l", [H // 2, 2 * E], f8,
                                       isOutput=False)

    # "Wq" carries the host-folded M = Wq @ Wk^T
    wq_d = nc.declare_dram_parameter("Wq", [H, H], f32, isOutput=False)
    wv8h_d = nc.declare_dram_parameter("Wv8h", [H // 2, 2 * H], f8,
                                       isOutput=False)
    wv8l_d = nc.declare_dram_parameter("Wv8l", [H // 2, 2 * H], f8,
                                       isOutput=False)
    # "bqc" carries the host-folded w = Wk @ bq, chunked [128, KC]
    bqc_d = nc.declare_dram_parameter("bqc", [128, KC], f32, isOutput=False)
    bvr_d = nc.declare_dram_parameter("bvr", [1, H], bf16, isOutput=False)
    onesc_d = nc.declare_dram_parameter("onesc", [128, 2], f32, isOutput=False)
    eye_d = nc.declare_dram_parameter("eye", [128, 128], bf16, isOutput=False)
    out_d = nc.declare_dram_parameter("out", [T, H], f32, isOutput=True)

    cp = ctx.enter_context(tc.tile_pool(name="cp", bufs=1))
    small = ctx.enter_context(tc.tile_pool(name="small", bufs=3))

    # ---- persistent SBUF tensors -----------------------------------------
    QT = cp.tile([128, KC * T], f32r, name="QT")    # A''^T hidden-major
    dke = cp.tile([128, KC * T], f32r, name="dke")  # per-chunk A''*x prods
    Vx8h = cp.tile([128, NE * H], f8, name="Vx8h")  # V_ext E-major fp8 hi
    Vx8l = cp.tile([128, NE * H], f8, name="Vx8l")  # and lo residual
    xT = cp.tile([128, KC * T], f32r, name="xT")    # chunk kc: cols [kc*T,+T)
    eT = cp.tile([128, KC * E], f32r, name="eT")
    NP = KC // 2  # DoubleRow kc-pairs
    Wv8h = cp.tile([128, NP * 2 * H], f8, name="Wv8h")  # 32*Wv hi/lo fp8
    Wv8l = cp.tile([128, NP * 2 * H], f8, name="Wv8l")
    eT8h = cp.tile([128, NP * 2 * E], f8, name="eT8h")
    eT8l = cp.tile([128, NP * 2 * E], f8, name="eT8l")
    xT8h = cp.tile([128, NP * 2 * T], f8, name="xT8h")
    xT8l = cp.tile([128, NP * 2 * T], f8, name="xT8l")
    ss_col = cp.tile([128, NT], f32, name="ss_col")    # s_self per tile
    ssm_col = cp.tile([128, NT], f32, name="ssm_col")  # ln128 - s_self
    bqc = cp.tile([128, KC], f32, name="bqc")       # holds w = Wk bq chunks
    bvr = cp.tile([1, H], bf16, name="bvr")
    bvb = cp.tile([128, H], bf16, name="bvb")
    ones_c = cp.tile([128, 2], f32r, name="ones_c")
    identb = cp.tile([128, 128], bf16, name="identb")

    loop_cm = tc.For_i(0, reps, 1) if reps > 1 else contextlib.nullcontext()
    with loop_cm:
      with tc.tile_pool(name="monoA", bufs=4, space="PSUM") as monoA, \
           tc.tile_pool(name="monoB", bufs=4, space="PSUM") as monoB:
        with tc.tile_pool(name="wq", bufs=4) as wq_pool:
            wqa = [wq_pool.tile([128, WT], f32r, name="wqa") for _ in range(KC)]
            wqb = [wq_pool.tile([128, WT], f32r, name="wqb") for _ in range(KC)]

            # ---- the ordered input stream (single queue = device order) --
            # wqa[0] and a 256-col slice of xT[0] lead, spread over three
            # queues so their fixed DMA latencies overlap: the first
            # matmul only waits on ~384KB.
            nc.sync.dma_start(wqa[0][:, 0:128],
                              wq_d[0:128, 0:128].bitcast(f32r))
            nc.scalar.dma_start(xT[:, 0:256],
                                xT_d[0:128, 0:256].bitcast(f32r))
            nc.gpsimd.dma_start(xT[:, 256:WT],
                                xT_d[0:128, 256:WT].bitcast(f32r))
            nc.sync.dma_start(wqa[0][:, 128:WT],
                              wq_d[0:128, 128:WT].bitcast(f32r))
            nc.sync.dma_start(xT[:, WT:T], xT_d[0:128, WT:T].bitcast(f32r))
            for kc in range(1, 5):
                nc.sync.dma_start(xT[:, kc * T:(kc + 1) * T],
                                  xT_d[kc * 128:(kc + 1) * 128, :].bitcast(f32r))
                nc.sync.dma_start(
                    wqa[kc][:],
                    wq_d[kc * 128:(kc + 1) * 128, 0:WT].bitcast(f32r))
                if kc == 2:
                    nc.sync.dma_start(bqc[:], bqc_d[:])
            # tail chunks: all first token-halves + weights stream first
            # (the matching matmuls are also emitted n0-across-chunks),
            # the second halves follow while those matmuls run
            for kc in range(5, KC):
                nc.sync.dma_start(
                    xT[:, kc * T:kc * T + WT],
                    xT_d[kc * 128:(kc + 1) * 128, 0:WT].bitcast(f32r))
                nc.sync.dma_start(
                    wqa[kc][:],
                    wq_d[kc * 128:(kc + 1) * 128, 0:WT].bitcast(f32r))
            for kc in range(5, KC):
                nc.sync.dma_start(
                    xT[:, kc * T + WT:(kc + 1) * T],
                    xT_d[kc * 128:(kc + 1) * 128, WT:T].bitcast(f32r))
            for kc in range(KC):
                if kc < 2:
                    # head chunks split: QB's first matmuls need only the
                    # leading 128-col slice of the weight chunk
                    nc.sync.dma_start(
                        wqb[kc][:, 0:128],
                        wq_d[kc * 128:(kc + 1) * 128,
                             WT:WT + 128].bitcast(f32r))
                    nc.sync.dma_start(
                        wqb[kc][:, 128:WT],
                        wq_d[kc * 128:(kc + 1) * 128,
                             WT + 128:H].bitcast(f32r))
                    continue
                nc.sync.dma_start(
                    wqb[kc][:],
                    wq_d[kc * 128:(kc + 1) * 128, WT:H].bitcast(f32r))
            nc.sync.dma_start(ones_c[:], onesc_d[:].bitcast(f32r))
            nc.sync.dma_start(bvr[:], bvr_d[:])
            nc.sync.dma_start(identb[:], eye_d[:])
            for pr in range(NP):
                nc.sync.dma_start(Wv8h[:, pr * 2 * H:(pr + 1) * 2 * H],
                                  wv8h_d[pr * 128:(pr + 1) * 128, :])
                nc.sync.dma_start(Wv8l[:, pr * 2 * H:(pr + 1) * 2 * H],
                                  wv8l_d[pr * 128:(pr + 1) * 128, :])
                nc.sync.dma_start(eT8h[:, pr * 2 * E:(pr + 1) * 2 * E],
                                  eT8h_d[pr * 128:(pr + 1) * 128, :])
                nc.sync.dma_start(eT8l[:, pr * 2 * E:(pr + 1) * 2 * E],
                                  eT8l_d[pr * 128:(pr + 1) * 128, :])
            for kc in range(KC):
                nc.sync.dma_start(eT[:, kc * E:(kc + 1) * E],
                                  eT_d[kc * 128:(kc + 1) * 128, :].bitcast(f32r))

            # preload the Exp activation table off the critical path
            dummy = small.tile([1, 2], f32, name="dummy")
            nc.scalar.memzero(dummy[:])
            nc.scalar.activation(dummy[:], dummy[:], Act.Exp, bias=0.0,
                                 scale=1.0)
            # bvb = bv broadcast to all partitions (Pool; no PE/PSUM)
            nc.gpsimd.partition_broadcast(bvb[:], bvr[:])
            # xT8 hi/lo fp8 split computed on-device (Act/Pool ride the
            # otherwise-idle window while xT streams in) instead of 2MB
            # of DMA; the psu matmuls then never race the DMA tail
            for kc in range(KC):
                o = (kc // 2) * 2 * T + (kc % 2) * T
                sl = slice(kc * T, (kc + 1) * T)
                nc.scalar.copy(xT8h[:, o:o + T], xT[:, sl])
                nc.gpsimd.tensor_tensor(xT8l[:, o:o + T], xT[:, sl],
                                        xT8h[:, o:o + T], Alu.subtract)

            # ---- QA / QB: A''^T = M^T @ xT (+w), kc-outer, half-ho -------
            # psq readout runs on the Act engine (Identity + per-partition
            # w bias); the dke product (A''^T (.) xT, feeds s_self) on DVE.
            for half, wqs in ((0, wqa), (1, wqb)):
                psq = ([monoA.tile([128, WT], f32, name="psqA", tag="acc")
                        for _ in range(4)]
                       + [monoB.tile([128, WT], f32, name="psqB", tag="acc")
                          for _ in range(4)])
                def a_mm(kc, n):
                    for ho4 in range(4):
                        nc.tensor.matmul(
                            psq[ho4 * NTW + n][:],
                            wqs[kc][:, ho4 * 128:(ho4 + 1) * 128],
                            xT[:, kc * T + n * WT: kc * T + (n + 1) * WT],
                            start=(kc == 0), stop=(kc == KC - 1))

                if half == 0:
                    # tail chunks n0-across-chunks first, matching the
                    # split DMA order: first halves + weights arrive
                    # first, the n1 halves stream behind the n0 matmuls
                    for kc in range(5):
                        for n in range(NTW):
                            a_mm(kc, n)
                    for n in range(NTW):
                        for kc in range(5, KC):
                            a_mm(kc, n)
                else:
                    for kc in range(KC):
                        for n in range(NTW):
                            a_mm(kc, n)
                # readout: each psq bank has ONE reader (the QT add, split
                # Act/DVE so the ladder halves and banks free fast, monoB
                # strips first so V0 unblocks early); the dke product then
                # reads QT from SBUF (DVE/Pool) without holding banks
                for s in (4, 5, 6, 7, 0, 1, 2, 3):
                    ho4, n = s // NTW, s % NTW
                    ho = half * 4 + ho4
                    sl = slice(ho * T + n * WT, ho * T + (n + 1) * WT)
                    if s % 2 == 0:
                        nc.scalar.add(QT[:, sl], psq[s][:],
                                      bqc[:, ho:ho + 1])
                    else:
                        nc.vector.tensor_scalar_add(QT[:, sl], psq[s][:],
                                                    bqc[:, ho:ho + 1])
                for s in (4, 5, 6, 7, 0, 1, 2, 3):
                    ho4, n = s // NTW, s % NTW
                    ho = half * 4 + ho4
                    sl = slice(ho * T + n * WT, ho * T + (n + 1) * WT)
                    eng = nc.vector if s % 2 == 0 else nc.gpsimd
                    eng.tensor_tensor(dke[:, sl], QT[:, sl], xT[:, sl],
                                      Alu.mult)

        # ---- V_ext: Vx = eT^T @ Wv via fp8 DoubleRow; Wv is pre-scaled
        # x32 on the host so its hi/lo fp8 split avoids e4m3 subnormals,
        # and the 1/32 is applied at readout.  3 cross terms; the lo*lo
        # term (~0.4%) is dropped.  eblk 0 runs right after QB; the ssp
        # column sums fill the gap before eblk 1. ------------------------
        def pair2(t, pr, width, lo, hi):
            return t[:, pr * 2 * width:(pr + 1) * 2 * width].rearrange(
                "p (two w) -> p two w", two=2)[:, :, lo:hi]

        VTERMS = ((eT8h, Wv8h), (eT8h, Wv8l), (eT8l, Wv8h))

        def v_ext_eblk(eblk, defer=None):
            psv = [monoB.tile([128, WH], f32, name="psv", tag="acc")
                   for _ in range(4)]
            for pr in range(NP):
                for ti, (lt, rt) in enumerate(VTERMS):
                    for e2 in range(2):
                        eo = eblk * 2 + e2
                        for n in range(NH):
                            nc.tensor.matmul(
                                psv[e2 * NH + n][:],
                                pair2(lt, pr, E, eo * 128, (eo + 1) * 128),
                                pair2(rt, pr, H, n * WH, (n + 1) * WH),
                                start=(pr == 0 and ti == 0),
                                stop=(pr == NP - 1 and ti == len(VTERMS) - 1),
                                perf_mode=DR)
            for e2 in range(2):
                eo = eblk * 2 + e2
                for n in range(NH):
                    s = e2 * NH + n
                    ph = Vx8h[:, eo * H + n * WH: eo * H + (n + 1) * WH]
                    pl = Vx8l[:, eo * H + n * WH: eo * H + (n + 1) * WH]
                    if defer is None:
                        nc.scalar.activation(ph, psv[s][:], Act.Copy,
                                             bias=0.0, scale=1.0 / 32)
                        nc.vector.scalar_tensor_tensor(
                            pl, psv[s][:], 1.0 / 32, ph,
                            Alu.mult, Alu.subtract)
                    else:
                        # decouple the fp8 split from the PSUM banks (so
                        # the release does not sit on the DVE queue right
                        # when tile 0's softmax chain needs it): a single
                        # Act copy per bank to an f32 SBUF stage, the
                        # hi/lo split deferred into the attention window
                        vb = vbs[s]
                        nc.scalar.mul(vb[:], psv[s][:], 1.0 / 32)
                        defer.append((vb, ph, pl))

        # s_self: column-sum dke via tiny matmuls, one PSUM tile per
        # (ho-half, token tile) so each accumulation group is a whole
        # tile; the early half only waits on its own readout ladder and
        # the halves are combined on DVE during V1.
        def ssp_half(h0, consume):
            for m in range(NT):
                ssp = monoA.tile([128, 2], f32, name="ssp", tag="acc")
                for kc in range(h0, h0 + KC // 2):
                    nc.tensor.matmul(
                        ssp[:],
                        dke[:, kc * T + m * 128:kc * T + (m + 1) * 128],
                        ones_c[:], start=(kc == h0),
                        stop=(kc == h0 + KC // 2 - 1))
                consume(m, ssp)

        ssp_half(0, lambda m, ssp: nc.vector.tensor_copy(
            ss_col[:, m:m + 1], ssp[:, 0:1]))

        v_ext_eblk(0)

        ssp_half(KC // 2, lambda m, ssp: nc.vector.tensor_tensor(
            ss_col[:, m:m + 1], ss_col[:, m:m + 1], ssp[:, 0:1], Alu.add))
        # ssm = ln128 - s_self (pre-folded for the per-tile softmax max)
        nc.vector.tensor_scalar(ssm_col[:], ss_col[:], -1.0, LN128,
                                Alu.mult, Alu.add)

        v_ext_eblk(1)

      # ---- attention per token tile ------------------------------------
      # v_tok runs UNSCALED (xTb @ Wv, no softmax dependency) in its own
      # PSUM groups, filling the PE while the softmax chain computes; the
      # p_self scaling is applied per-partition at readout.
      with tc.tile_pool(name="ps_att", bufs=2, space="PSUM") as ps_att, \
           tc.tile_pool(name="ps_tr", bufs=2, space="PSUM") as ps_tr, \
           tc.tile_pool(name="ps_cu", bufs=1, space="PSUM") as ps_cu, \
           tc.tile_pool(name="work_a", bufs=4) as work_a, \
           tc.tile_pool(name="pt8", bufs=2) as pt8_pool:
        for m in range(NT):
            last = m == NT - 1
            # s_ext = A''^T.T @ eT  -> [128 tokens, E]  (f32r)
            ps_s = ps_att.tile([128, E], f32, name="ps_s")
            for kc in range(KC):
                nc.tensor.matmul(
                    ps_s[:],
                    QT[:, kc * T + m * 128: kc * T + (m + 1) * 128],
                    eT[:, kc * E:(kc + 1) * E],
                    start=(kc == 0), stop=(kc == KC - 1))

            nmx = small.tile([128, 1], f32, name="nmx")
            nc.vector.tensor_reduce(nmx[:], ps_s[:], axis=X, op=Alu.max,
                                    negate=True)
            # nmx2b = min(ln128 - ss, ln128 - max(s_ext))
            #       = ln128 - max(ss, max(s_ext))
            nmx2b = small.tile([128, 1], f32, name="nmx2b")
            nc.vector.scalar_tensor_tensor(
                nmx2b[:], nmx[:], LN128, ssm_col[:, m:m + 1],
                Alu.add, Alu.min)

            # probs are computed x128 (bias includes ln128) so their fp8
            # hi/lo split stays clear of e4m3 subnormals; the 1/128 is
            # carried by r' = 1/(128 Z).  exp runs in two 256-col halves
            # so the transposes/fp8-split/ctx start on half the tile.
            pe = work_a.tile([128, E], bf16, name="pe")
            Ze = small.tile([128, 1], f32, name="Ze")
            nc.scalar.activation(pe[:], ps_s[:], Act.Exp, bias=nmx2b[:],
                                 scale=1.0, accum_out=Ze[:])
            # P' = 128 * p_self
            p128 = small.tile([128, 1], f32, name="p128")
            nc.scalar.activation(p128[:], ss_col[:, m:m + 1],
                                 Act.Exp, bias=nmx2b[:], scale=1.0)
            Zt = small.tile([128, 1], f32, name="Zt")
            nc.vector.tensor_tensor(Zt[:], Ze[:], p128[:], Alu.add)
            rp = small.tile([128, 1], f32, name="rp")
            nc.vector.reciprocal(rp[:], Zt[:])
            pr = small.tile([128, 1], f32, name="pr")
            nc.vector.scalar_tensor_tensor(pr[:], p128[:], 1.0 / 32,
                                           rp[:], Alu.mult, Alu.mult)

            # unscaled v_tok: psu[n] = (xTb slice).T @ Wv — independent of
            # the softmax, keeps the PE busy during the chain above.
            # The probability transposes are emitted MID-psu (exp is
            # ready by then) so the trailing psu matmuls cover the fp8
            # conversion latency instead of the PE idling before ctx.
            psu = [ps_cu.tile([128, WH], f32, name=f"psu{n}",
                              tag=f"u{n}") for n in range(NH)]
            pst = ps_tr.tile([128, NE * 128], bf16, name="pst")
            UTERMS = ((xT8h, Wv8h), (xT8h, Wv8l), (xT8l, Wv8h))
            for pr8 in range(NP):
                for ti, (lt, rt) in enumerate(UTERMS):
                    if pr8 == 2 and ti == 2:
                        # transpose unnormalized ext probs into ONE PSUM
                        # tile (disjoint column slices run back to back)
                        for ec in range(NE):
                            nc.tensor.transpose(
                                pst[:, ec * 128:(ec + 1) * 128],
                                pe[:, ec * 128:(ec + 1) * 128],
                                identb[:])
                    lhsT = pair2(lt, pr8, T, m * 128, (m + 1) * 128)
                    for n in range(NH):
                        nc.tensor.matmul(
                            psu[n][:], lhsT,
                            pair2(rt, pr8, H, n * WH, (n + 1) * WH),
                            start=(pr8 == 0 and ti == 0),
                            stop=(pr8 == NP - 1 and
                                  ti == len(UTERMS) - 1),
                            perf_mode=DR)
            # (GPSIMD cannot read PSUM on hardware: hi split on Act, lo
            # residual on DVE with the single allowed PSUM operand; the
            # last tile puts both on DVE so its ctx is not stuck behind
            # the Act queue)
            Pt8h = pt8_pool.tile([128, NE * 128], f8, name="Pt8h")
            Pt8l = pt8_pool.tile([128, NE * 128], f8, name="Pt8l")
            (nc.vector.tensor_copy if last else nc.scalar.copy)(
                Pt8h[:], pst[:])
            nc.vector.tensor_tensor(Pt8l[:], pst[:], Pt8h[:], Alu.subtract)

            # ctx_ext = Pt.T @ Vx
            psc = [ps_cu.tile([128, WH], f32, name=f"psc{n}", tag=f"c{n}")
                   for n in range(NH)]
            CTERMS = ((Pt8h, Vx8h), (Pt8h, Vx8l), (Pt8l, Vx8h))
            NEP = NE // 2  # ec-pairs

            # (ep, ti) visit order: the h0-fed groups first, the Pt8l-fed
            # term (ti=2) last, matching when each fp8 half lands
            CTX_ORDER = ((0, 0), (0, 1), (1, 0), (1, 1), (0, 2), (1, 2))

            def ctx_mm(n, ep, ti, start, stop):
                lt, rt = CTERMS[ti]
                nc.tensor.matmul(
                    psc[n][:],
                    lt[:].rearrange("p (ep two e) -> p ep two e",
                                    ep=NEP, two=2)[:, ep],
                    rt[:, 2 * ep * H:(2 * ep + 2) * H].rearrange(
                        "p (two h) -> p two h",
                        two=2)[:, :, n * WH:(n + 1) * WH],
                    start=start, stop=stop, perf_mode=DR)

            if not last:
                for i, (ep, ti) in enumerate(CTX_ORDER):
                    for n in range(NH):
                        ctx_mm(n, ep, ti, i == 0, i == len(CTX_ORDER) - 1)
            else:
                # finish chunk 1 first so its readout+store overlaps
                # chunk 0's matmuls and the drain is one chunk shorter
                for n in (1, 0):
                    for i, (ep, ti) in enumerate(CTX_ORDER):
                        ctx_mm(n, ep, ti, i == 0, i == len(CTX_ORDER) - 1)

            # out = r'*ctx_ext + (P'*r'/32)*v_tok + bvb, stored in strips
            order = (1, 0) if last else tuple(range(NH))
            osbs = {}
            # psu stops well before ctx_ext: fold it early, split as
            # Act (x pr, per-partition scale) + Pool (+bvb, all-SBUF),
            # keeping DVE free for the ctx folds
            for n in order:
                osbs[n] = work_a.tile([128, WH], f32, name="osb")
                nc.scalar.activation(osbs[n][:], psu[n][:], Act.Copy,
                                     bias=0.0, scale=pr[:])
                nc.gpsimd.tensor_tensor(
                    osbs[n][:], osbs[n][:],
                    bvb[:, n * WH:(n + 1) * WH], Alu.add)
            for n in order:
                if last and n == 0:
                    # split the final strip so the last store is smaller
                    # and launches off a shorter fold
                    for c0, c1, q in ((128, 512, nc.scalar),
                                      (0, 128, nc.sync)):
                        nc.vector.scalar_tensor_tensor(
                            osbs[n][:, c0:c1], psc[n][:, c0:c1],
                            rp[:], osbs[n][:, c0:c1],
                            Alu.mult, Alu.add)
                        q.dma_start(
                            out_d[m * 128:(m + 1) * 128, c0:c1],
                            osbs[n][:, c0:c1])
                    continue
                nc.vector.scalar_tensor_tensor(
                    osbs[n][:], psc[n][:],
                    rp[:], osbs[n][:],
                    Alu.mult, Alu.add)
                if last:
                    nc.sync.dma_start(
                        out_d[m * 128:(m + 1) * 128,
                              n * WH:n * WH + 256], osbs[n][:, 0:256])
                    nc.scalar.dma_start(
                        out_d[m * 128:(m + 1) * 128,
                              n * WH + 256:(n + 1) * WH],
                        osbs[n][:, 256:512])
                else:
                    nc.scalar.dma_start(
                        out_d[m * 128:(m + 1) * 128, n * WH:(n + 1) * WH],
                        osbs[n][:])


def _build_module(T, H, E, reps=1):
    from contextlib import ExitStack
    import concourse.tile as tile
    from concourse import bacc

    nc = bacc.Bacc(None)
    with ExitStack() as ctx:
        tc = ctx.enter_context(tile.TileContext(nc))
        _emit(nc, tc, ctx, T, H, E, reps)
    nc.finalize()
    return nc


# --------------------------------------------------------------------------
# host side
# --------------------------------------------------------------------------

def _shard_inputs(hidden_states, external_embeddings, Wq, bq, Wk, bk, Wv, bv):
    """Build the per-core input maps (host-side layout prep)."""
    hs = np.asarray(hidden_states, dtype=np.float32)
    ext = np.asarray(external_embeddings, dtype=np.float32)
    Wq64 = np.asarray(Wq, dtype=np.float64)
    Wk64 = np.asarray(Wk, dtype=np.float64)
    Wv = np.asarray(Wv, dtype=np.float32)
    bq = np.asarray(bq, dtype=np.float64)
    bv = np.asarray(bv, dtype=np.float32)

    # score-path weight folding (see module docstring): the device only
    # ever sees M = Wq Wk^T (as "Wq") and w = Wk bq (as "bqc"); the
    # per-token shift hs.(Wq bk) and the constant bq.bk cancel in softmax.
    M = np.ascontiguousarray((Wq64 @ Wk64.T).astype(np.float32))
    wbq = (Wk64 @ bq).astype(np.float32)

    f8 = ml_dtypes.float8_e4m3

    def fp8_pairs(a):
        """[H, N] f32 -> hi/lo fp8 arrays [H//2, 2*N] in DoubleRow
        kc-pair layout: row pr*128+p holds chunks (2pr, 2pr+1)."""
        Hd, N = a.shape
        hi = a.astype(f8)
        lo = (a - hi.astype(np.float32)).astype(f8)
        out = []
        for arr in (hi, lo):
            v = arr.reshape(Hd // 256, 2, 128, N).transpose(0, 2, 1, 3)
            out.append(np.ascontiguousarray(v.reshape(Hd // 2, 2 * N)))
        return out

    Wv8h, Wv8l = fp8_pairs(Wv * 32.0)

    KC = H // 128
    bqc = np.ascontiguousarray(wbq.reshape(KC, 128).T)  # [128, KC]
    bvr = np.ascontiguousarray(bv.reshape(1, H).astype(ml_dtypes.bfloat16))

    flat = hs.reshape(B * S, H)
    in_maps = []
    _ET8 = {}
    for c in range(NCORES):
        b = (c * T) // S
        xT = np.ascontiguousarray(flat[c * T:(c + 1) * T, :].T)  # [H, T]
        eT = np.ascontiguousarray(ext[b].T)                      # [H, E]
        eT8h, eT8l = _ET8.setdefault(b, fp8_pairs(eT))
        in_maps.append({
            "xT": xT,
            "eT": eT, "eT8h": eT8h, "eT8l": eT8l,
            "Wq": M, "Wv8h": Wv8h, "Wv8l": Wv8l,
            "bqc": bqc, "bvr": bvr,
            "onesc": _ONESC, "eye": _EYEB,
        })
    return in_maps


def kernel(hidden_states, external_embeddings, Wq, bq, Wk, bk, Wv, bv):
    from concourse.bass_utils import run_bass_kernel_spmd

    key = "main"
    if key not in _RUNNER_CACHE:
        _RUNNER_CACHE[key] = _build_module(T, H, E)
    nc = _RUNNER_CACHE[key]

    in_maps = _shard_inputs(hidden_states, external_embeddings,
                            Wq, bq, Wk, bk, Wv, bv)
    res = run_bass_kernel_spmd(nc, in_maps, list(range(NCORES)))
    out = np.concatenate([res.results[c]["out"] for c in range(NCORES)],
                         axis=0)
    return out.reshape(B, S, H)
